# revision 32
# baseline (speedup 1.0000x reference)
"""AttentionHead kernel for 8 Trainium2 NeuronCores.

Problem (per sample, B=4): x:[256,64,64] -> q/k/v 1x1-conv projections
(+positional encoding on q,k), S = q^T k / 8, softmax over the QUERY axis,
out = attn @ v, then 1x1-conv MLP with Mish + residual.

Sharding: 2 cores per sample, split over the query axis i (2048 queries each).
Softmax normalizes over i, so the per-key denominator den[j] = sum_i exp(S[i,j])
needs one tiny AllReduce per core pair (done in 2 chunks; each chunk's latency
hides behind compute); den then folds into v (v/den), everything else is local,
and the output halves are disjoint.

Layout trick: compute S transposed, S[j,i] = (k^T q)[j,i], keys j on partitions.
exp runs PSUM->SBUF with a per-partition accumulate (the denominator for free),
and exp(S)[j,i] is then directly the correct operand layout for both
out[c,i] = sum_j v[c,j]*attnT[j,i] and the MLP — zero on-device transposes.
All matmul operands bf16 (fp32 PSUM accumulation): ~4e-5 rel err.

Bias handling: q/k biases are folded into the positional-encoding tensors on
the host; the v bias is a broadcast tensor added during the PSUM->SBUF move;
b1 rides the Mish activation's per-partition bias; b2 rides the residual add.
Mish = x*tanh(softplus(x)): sp via the Softplus LUT on HW (or exp+ln when
use_softplus=False, for CoreSim), then Tanh, then one DVE multiply.

Input DMAs are merged into a few big transfers (issue cost on the sequencer is
~650ns each) and split across the sync and gpsimd queues.
"""

import numpy as np
import ml_dtypes

import concourse.bass as bass
import concourse.bacc as bacc
import concourse.mybir as mybir
import concourse.tile as tile
from concourse.tile_rust import add_dep_helper

BF16 = mybir.dt.bfloat16
F32 = mybir.dt.float32
AF = mybir.ActivationFunctionType
OP = mybir.AluOpType
bf16 = ml_dtypes.bfloat16

B, C, H, W = 4, 256, 64, 64
N = H * W            # 4096 pixels
QK = 64
IS = N // 2          # 2048 queries per core
NJT = N // 128       # 32 key tiles
NIB = IS // 512      # 4 i-blocks
NCH = 4              # den allreduce chunks
JCH = NJT // NCH     # 16 key tiles per chunk
N_CORES = 8
REPLICA_GROUPS = [[0, 1], [2, 3], [4, 5], [6, 7]]


def build_program(n_cores: int = N_CORES, enable_asserts: bool = False,
                  use_softplus: bool = False) -> bass.Bass:
    nc = bacc.Bacc(
        "TRN2",
        target_bir_lowering=False,
        debug=False,
        enable_asserts=enable_asserts,
        num_devices=n_cores,
    )

    # Per-core inputs (data differs by core; program is identical).
    # xq/xb/xf hold the two 128-row channel halves side by side:
    # [:, kt*COLS : (kt+1)*COLS] is channel rows kt*128..kt*128+127.
    xq_d = nc.dram_tensor("xq", [128, 2 * IS], BF16, kind="ExternalInput").ap()
    xb_d = nc.dram_tensor("xb", [128, 2 * N], BF16, kind="ExternalInput").ap()
    xf_d = nc.dram_tensor("xf", [128, 2 * IS], F32, kind="ExternalInput").ap()
    pe1q_d = nc.dram_tensor("pe1q", [QK, IS], BF16, kind="ExternalInput").ap()
    # Shared weights (same on all cores).
    pe1_d = nc.dram_tensor("pe1", [QK, N], BF16, kind="ExternalInput").ap()
    wqk_d = nc.dram_tensor("wqk", [128, 256], BF16, kind="ExternalInput").ap()
    # wmlp = wvt | w1t | w2t | bvb
    wmlp_d = nc.dram_tensor("wmlp", [128, 1792], BF16, kind="ExternalInput").ap()
    bcols_d = nc.dram_tensor("bcols", [128, 4], F32, kind="ExternalInput").ap()

    y_d = nc.dram_tensor("y", [C, IS], F32, kind="ExternalOutput").ap()

    with tile.TileContext(nc) as tc:
        with (
            tc.tile_pool(name="const", bufs=1) as cpool,
            tc.tile_pool(name="qk", bufs=1) as qkpool,
            tc.tile_pool(name="outsb", bufs=1) as outpool,
            tc.tile_pool(name="den", bufs=1) as denpool,
            tc.tile_pool(name="dram", bufs=1, space="DRAM") as dram,
        ):
            # q-projection inputs first (gate the whole pipeline), then the
            # rest; bulky non-urgent loads go on the gpsimd queue.
            wqk_sb = cpool.tile([128, 256], BF16)
            nc.sync.dma_start(wqk_sb[:], wqk_d[:])

            q_sb = qkpool.tile([QK, IS], BF16)     # q, d on partitions
            k_sb = qkpool.tile([QK, N], BF16)      # k, d on partitions
            vtpool = tc.alloc_tile_pool(name="vt", bufs=1, side="right")
            vt_sb = vtpool.tile([128, NJT * 256], BF16)   # v^T, j on partitions
            den_sb = denpool.tile([128, NJT], F32)
            dsum_sb = denpool.tile([128, NJT], F32)
            rden_sb = denpool.tile([128, NJT], F32)
            out_sb = [outpool.tile([128, IS], BF16, name=f"out_sb{i}") for i in range(2)]

            # ---- Phase 1 + early S: projections interleaved with the first
            # NBOOT S-tiles (computed in [128,1024] PSUM halves so they fit
            # alongside the projection PSUM pools) to start ScalarE's exp
            # stream as early as possible. ----
            NBOOT = 10
            with tc.tile_pool(name="attn", bufs=1) as apool:
                attn_sb = apool.tile([128, NJT * IS], BF16)   # 16 MiB
                with (
                    tc.tile_pool(name="xq", bufs=1) as xqpool,
                    tc.tile_pool(name="xb", bufs=1) as xbpool,
                    tc.tile_pool(name="pe", bufs=1) as pepool,
                    tc.tile_pool(name="psA", bufs=2, space="PSUM") as psA,
                    tc.tile_pool(name="psV", bufs=2, space="PSUM") as psV,
                    tc.tile_pool(name="psS0", bufs=2, space="PSUM") as psS0,
                ):
                    xq_sb = xqpool.tile([128, 2 * IS], BF16)
                    xb_sb = xbpool.tile([128, 2 * N], BF16)
                    pe1q_sb = pepool.tile([QK, IS], BF16)
                    pe1_sb = pepool.tile([QK, N], BF16)
                    for ch in range(2):
                        nc.sync.dma_start(xq_sb[:, bass.ts(ch, IS)],
                                          xq_d[:, bass.ts(ch, IS)])
                    nc.sync.dma_start(pe1q_sb[:], pe1q_d[:])
                    for ch in range(4):
                        nc.sync.dma_start(xb_sb[:, bass.ts(ch, N // 2)],
                                          xb_d[:, bass.ts(ch, N // 2)])
                    for ch in range(2):
                        nc.sync.dma_start(pe1_sb[:, bass.ts(ch, N // 2)],
                                          pe1_d[:, bass.ts(ch, N // 2)])
                    wmlp_sb = cpool.tile([128, 1792], BF16)
                    bcols_sb = cpool.tile([128, 4], F32)
                    nc.gpsimd.dma_start(wmlp_sb[:], wmlp_d[:])
                    nc.gpsimd.dma_start(bcols_sb[:], bcols_d[:])
                    wvt = wmlp_sb[:, 0:512]
                    w1t = wmlp_sb[:, 512:1024]
                    w2t = wmlp_sb[:, 1024:1536]
                    bvb = wmlp_sb[:, 1536:1792]
                    b1c = bcols_sb[:, 0:2]
                    b2c = bcols_sb[:, 2:4]

                    den_h = denpool.tile([128, 2 * NBOOT], F32)

                    def q_proj(ib):
                        sl = bass.ts(ib, 512)
                        ps = psA.tile([QK, 512], F32, name="psa")
                        for kt in range(2):
                            nc.tensor.matmul(ps[:], wqk_sb[:, bass.ts(kt, QK)],
                                             xq_sb[:, ib * 1024 + kt * 512:
                                                   ib * 1024 + (kt + 1) * 512],
                                             start=(kt == 0), stop=(kt == 1))
                        nc.vector.tensor_add(q_sb[:, sl], ps[:], pe1q_sb[:, sl])

                    def k_proj(jb):
                        sl = bass.ts(jb, 512)
                        ps = psA.tile([QK, 512], F32, name="psa")
                        for kt in range(2):
                            nc.tensor.matmul(ps[:], wqk_sb[:, 128 + kt * QK:
                                                           128 + (kt + 1) * QK],
                                             xb_sb[:, kt * N + jb * 512:
                                                   kt * N + (jb + 1) * 512],
                                             start=(kt == 0), stop=(kt == 1))
                        nc.vector.tensor_add(k_sb[:, sl], ps[:], pe1_sb[:, sl])

                    def s_boot(jt):
                        for h2 in range(2):
                            ps0 = psS0.tile([128, 1024], F32, name="pss0")
                            for n2 in range(2):
                                ib = h2 * 2 + n2
                                nc.tensor.matmul(ps0[:, bass.ts(n2, 512)],
                                                 k_sb[:, bass.ts(jt, 128)],
                                                 q_sb[:, bass.ts(ib, 512)],
                                                 start=True, stop=True)
                            nc.scalar.activation(
                                attn_sb[:, jt * IS + h2 * 1024:
                                        jt * IS + (h2 + 1) * 1024],
                                ps0[:], AF.Exp, scale=0.125,
                                accum_out=den_h[:, h2 * NBOOT + jt:
                                                h2 * NBOOT + jt + 1])

                    def v_proj(jt):
                        ps = psV.tile([128, 256], F32, name="psv")
                        for kt in range(2):
                            nc.tensor.matmul(ps[:],
                                             xb_sb[:, kt * N + jt * 128:
                                                   kt * N + (jt + 1) * 128],
                                             wvt[:, bass.ts(kt, 256)],
                                             start=(kt == 0), stop=(kt == 1))
                        nc.vector.tensor_add(vt_sb[:, bass.ts(jt, 256)],
                                             ps[:], bvb[:])

                    for ib in range(NIB):
                        q_proj(ib)
                    k_proj(0)
                    k_proj(1)
                    for jt in range(4):
                        s_boot(jt)
                    k_proj(2)
                    for jt in range(4, 8):
                        s_boot(jt)
                    k_proj(3)
                    for jt in range(8, NBOOT):
                        s_boot(jt)
                    for jb in range(4, N // 512):
                        k_proj(jb)
                    for jt in range(NJT):
                        v_proj(jt)
                    nc.vector.tensor_add(den_sb[:, 0:NBOOT],
                                         den_h[:, 0:NBOOT],
                                         den_h[:, NBOOT:2 * NBOOT])

                # ---- Phase 2: remaining S[j,i] = (k^T q)/8, attnT = exp(S) ----
                with tc.tile_pool(name="psS", bufs=2, space="PSUM") as psS:
                    for jt in range(NBOOT, NJT):
                        ps = psS.tile([128, IS], F32)
                        for ib in range(NIB):
                            nc.tensor.matmul(ps[:, bass.ts(ib, 512)],
                                             k_sb[:, bass.ts(jt, 128)],
                                             q_sb[:, bass.ts(ib, 512)],
                                             start=True, stop=True)
                        nc.scalar.activation(attn_sb[:, bass.ts(jt, IS)], ps[:],
                                             AF.Exp, scale=0.125,
                                             accum_out=den_sb[:, jt:jt + 1])

                # den AllReduce within the core pair (2 chunks, latency hidden),
                # then fold 1/den into v^T
                vtspool = tc.alloc_tile_pool(name="vts", bufs=1)
                vts_sb = vtspool.tile([128, NJT * 256], BF16)  # v^T / den
                for ch in range(NCH):
                    csl = bass.ts(ch, JCH)
                    den_in = dram.tile([128, JCH], F32, name=f"den_in{ch}")
                    den_out = dram.tile([128, JCH], F32, name=f"den_out{ch}")
                    nc.sync.dma_start(den_in[:], den_sb[:, csl])
                    nc.gpsimd.collective_compute(
                        "AllReduce", OP.add,
                        replica_groups=REPLICA_GROUPS,
                        ins=[den_in.opt()], outs=[den_out.opt()],
                    )
                    nc.sync.dma_start(dsum_sb[:, csl], den_out[:])
                    nc.vector.reciprocal(rden_sb[:, csl], dsum_sb[:, csl])
                    for jt in range(ch * JCH, (ch + 1) * JCH):
                        nc.vector.tensor_scalar_mul(vts_sb[:, bass.ts(jt, 256)],
                                                    vt_sb[:, bass.ts(jt, 256)],
                                                    rden_sb[:, jt:jt + 1])
                vtpool.release()

                # ---- Phase 3: out[c,i] = sum_j vts[j,c] * attnT[j,i] ----
                # Two j-half visits so the second den chunk's allreduce hides
                # behind the first half's matmuls.
                # Phase 4's matmuls reuse the 8 out-accumulator PSUM banks
                # (Tile serializes on the read->overwrite dependencies), and
                # visit 2 runs ib-major so each i-block's MLP front (W1 matmul,
                # mish exp/ln on ScalarE) overlaps the remaining attn@v work.
                with (
                    tc.tile_pool(name="xf", bufs=1) as xfpool,
                    tc.tile_pool(name="h", bufs=1) as hpool,
                    tc.tile_pool(name="mtmp", bufs=1) as mpool,
                    tc.tile_pool(name="y", bufs=2) as ypool,
                    tc.tile_pool(name="psO", bufs=1, space="PSUM") as psO,
                ):
                    xf_sb = xfpool.tile([128, 2 * IS], F32)
                    nc.gpsimd.dma_start(xf_sb[:], xf_d[:])
                    h_sb = [hpool.tile([128, IS], BF16, name=f"h_sb{i}")
                            for i in range(2)]
                    pso = {}
                    for mt in range(2):
                        for ib in range(NIB):
                            pso[mt, ib] = psO.tile([128, 512], F32,
                                                   name=f"pso{mt}{ib}")

                    def av_mms(mt, ib, jlo, jhi):
                        for jt in range(jlo, jhi):
                            nc.tensor.matmul(
                                pso[mt, ib][:],
                                vts_sb[:, jt * 256 + mt * 128:
                                       jt * 256 + (mt + 1) * 128],
                                attn_sb[:, jt * IS + ib * 512:
                                        jt * IS + (ib + 1) * 512],
                                start=(jt == 0), stop=(jt == NJT - 1),
                                skip_group_check=True)

                    sp_ts = {}
                    mish_exps, mish_lns = [], []
                    for v in range(NCH - 1):
                        for mt in range(2):
                            for ib in range(NIB):
                                av_mms(mt, ib, v * JCH, (v + 1) * JCH)
                    for ib in range(NIB):
                        for mt in range(2):
                            av_mms(mt, ib, (NCH - 1) * JCH, NJT)
                        for mt in range(2):
                            nc.vector.tensor_copy(
                                out_sb[mt][:, bass.ts(ib, 512)], pso[mt, ib][:])
                        # MLP front for this i-block (reuses the freed banks);
                        # exp lands in h_sb's storage (dead until the final
                        # mish multiply overwrites it)
                        for mt in range(2):
                            ps = pso[mt, ib]
                            for kt in range(2):
                                nc.tensor.matmul(
                                    ps[:],
                                    w1t[:, kt * 256 + mt * 128:
                                        kt * 256 + (mt + 1) * 128],
                                    out_sb[kt][:, bass.ts(ib, 512)],
                                    start=(kt == 0), stop=(kt == 1),
                                    skip_group_check=True)
                            mish_exps.append(nc.scalar.activation(
                                h_sb[mt][:, bass.ts(ib, 512)], ps[:], AF.Exp,
                                bias=b1c[:, mt:mt + 1]))
                    # softplus: sp = ln(1+e); one wide inst per channel
                    # half so the table set switches only twice overall
                    for mt in range(2):
                        sp_t = mpool.tile([128, IS], BF16,
                                          name=f"mish_sp{mt}", bufs=1)
                        ln_i = nc.scalar.activation(sp_t[:], h_sb[mt][:],
                                                    AF.Ln, bias=1.0)
                        for e in mish_exps:
                            add_dep_helper(ln_i.ins, e.ins, sync=False,
                                           reason="batch act table sets")
                        mish_lns.append(ln_i)
                        sp_ts[mt] = sp_t

                    # tanh batch (one table switch), then
                    # h = (h_psum+b1)*tanh(sp) per block
                    for mt in range(2):
                        # tanh overwrites h_sb (the exp values are dead)
                        th_i = nc.scalar.activation(h_sb[mt][:], sp_ts[mt][:],
                                                    AF.Tanh)
                        for l in mish_lns:
                            add_dep_helper(th_i.ins, l.ins, sync=False,
                                           reason="batch act table sets")
                    for ib in range(NIB):
                        for mt in range(2):
                            nc.vector.scalar_tensor_tensor(
                                h_sb[mt][:, bass.ts(ib, 512)], pso[mt, ib][:],
                                b1c[:, mt:mt + 1], h_sb[mt][:, bass.ts(ib, 512)],
                                op0=OP.add, op1=OP.mult)
                    for ib in range(NIB):
                        for mt in range(2):
                            sl = bass.ts(ib, 512)
                            ps = pso[mt, ib]
                            for kt in range(2):
                                nc.tensor.matmul(
                                    ps[:],
                                    w2t[:, kt * 256 + mt * 128:
                                        kt * 256 + (mt + 1) * 128],
                                    h_sb[kt][:, sl],
                                    start=(kt == 0), stop=(kt == 1),
                                    skip_group_check=True)
                            y_sb = ypool.tile([128, 512], F32)
                            nc.vector.scalar_tensor_tensor(
                                y_sb[:], ps[:], b2c[:, mt:mt + 1],
                                xf_sb[:, mt * IS + ib * 512:
                                      mt * IS + (ib + 1) * 512],
                                op0=OP.add, op1=OP.add)
                            eng = nc.sync if mt == 0 else nc.scalar
                            eng.dma_start(
                                y_d[mt * 128:(mt + 1) * 128, sl], y_sb[:])
                vtspool.release()
    nc.finalize()
    return nc


def _to_lhsT_sb(w):
    """[256, M] fp32 -> SBUF layout [128, 2*M] bf16: col block kt holds rows
    kt*128..kt*128+127 of w."""
    k, m = w.shape
    assert k == 256
    return np.ascontiguousarray(
        w.reshape(2, 128, m).transpose(1, 0, 2).reshape(128, 2 * m).astype(bf16))


def _bf(a):
    return np.ascontiguousarray(np.asarray(a, dtype=np.float32).astype(bf16))


def _halves(a):
    """[256, X] -> [128, 2*X] with the two 128-row halves side by side."""
    return np.ascontiguousarray(np.concatenate([a[:128], a[128:]], axis=1))


def make_in_maps(x, WQ, bQ, WK, bK, WV, bV, PE, W1, b1, W2, b2, n_cores=N_CORES):
    x = np.asarray(x, dtype=np.float32)
    xf3 = np.ascontiguousarray(x.reshape(B, C, N))
    pef = np.asarray(PE, dtype=np.float32).reshape(QK, N)
    pe1 = _bf(pef + np.asarray(bK, np.float32)[:, None])
    pe1q_full = _bf(pef + np.asarray(bQ, np.float32)[:, None])

    wq = _to_lhsT_sb(np.asarray(WQ, np.float32).T)   # [128, 128]
    wk = _to_lhsT_sb(np.asarray(WK, np.float32).T)
    wmlp = np.concatenate([
        _to_lhsT_sb(np.asarray(WV, np.float32).T),
        _to_lhsT_sb(np.asarray(W1, np.float32).T),
        _to_lhsT_sb(np.asarray(W2, np.float32).T),
        np.broadcast_to(_bf(np.asarray(bV)[None, :]), (128, 256)),
    ], axis=1)
    bcols = np.concatenate([
        np.asarray(b1, np.float32).reshape(2, 128).T,
        np.asarray(b2, np.float32).reshape(2, 128).T,
    ], axis=1)

    shared = {
        "pe1": pe1,
        "wqk": np.ascontiguousarray(np.concatenate([wq, wk], axis=1)),
        "wmlp": np.ascontiguousarray(wmlp),
        "bcols": np.ascontiguousarray(bcols),
    }
    in_maps = []
    for core in range(n_cores):
        s, h = core // 2, core % 2
        isl = slice(h * IS, (h + 1) * IS)
        xb = _bf(xf3[s])
        m = dict(shared)
        m["xb"] = _halves(xb)
        # xq is ib-interleaved: [:, ib*1024+kt*512 : ...] = channel-half kt,
        # query block ib — so the first DMA chunk covers ib 0-1 completely.
        xqs = xb[:, isl]
        m["xq"] = np.ascontiguousarray(np.concatenate(
            [np.concatenate([xqs[:128, ib * 512:(ib + 1) * 512],
                             xqs[128:, ib * 512:(ib + 1) * 512]], axis=1)
             for ib in range(NIB)], axis=1))
        m["xf"] = _halves(xf3[s][:, isl])
        m["pe1q"] = np.ascontiguousarray(pe1q_full[:, isl])
        in_maps.append(m)
    return in_maps


def assemble_output(results, n_cores=N_CORES):
    y = np.empty((B, C, N), dtype=np.float32)
    for s in range(B):
        y[s][:, :IS] = results[2 * s]["y"]
        y[s][:, IS:] = results[2 * s + 1]["y"]
    return y.reshape(B, C, H, W)


_PROG = None


def kernel(**inputs) -> np.ndarray:
    global _PROG
    from concourse.bass_utils import run_bass_kernel_spmd
    if _PROG is None:
        _PROG = build_program(N_CORES)
    in_maps = make_in_maps(**inputs)
    res = run_bass_kernel_spmd(_PROG, in_maps, core_ids=list(range(N_CORES)))
    return assemble_output(res.results)


# revision 33
# speedup vs baseline: 1.0094x; 1.0094x over previous
"""AttentionHead kernel for 8 Trainium2 NeuronCores.

Problem (per sample, B=4): x:[256,64,64] -> q/k/v 1x1-conv projections
(+positional encoding on q,k), S = q^T k / 8, softmax over the QUERY axis,
out = attn @ v, then 1x1-conv MLP with Mish + residual.

Sharding: 2 cores per sample, split over the query axis i (2048 queries each).
Softmax normalizes over i, so the per-key denominator den[j] = sum_i exp(S[i,j])
needs one tiny AllReduce per core pair (done in 2 chunks; each chunk's latency
hides behind compute); den then folds into v (v/den), everything else is local,
and the output halves are disjoint.

Layout trick: compute S transposed, S[j,i] = (k^T q)[j,i], keys j on partitions.
exp runs PSUM->SBUF with a per-partition accumulate (the denominator for free),
and exp(S)[j,i] is then directly the correct operand layout for both
out[c,i] = sum_j v[c,j]*attnT[j,i] and the MLP — zero on-device transposes.
All matmul operands bf16 (fp32 PSUM accumulation): ~4e-5 rel err.

Bias handling: q/k biases are folded into the positional-encoding tensors on
the host; the v bias is a broadcast tensor added during the PSUM->SBUF move;
b1 rides the Mish activation's per-partition bias; b2 rides the residual add.
Mish = x*tanh(softplus(x)): sp via the Softplus LUT on HW (or exp+ln when
use_softplus=False, for CoreSim), then Tanh, then one DVE multiply.

Input DMAs are merged into a few big transfers (issue cost on the sequencer is
~650ns each) and split across the sync and gpsimd queues.
"""

import numpy as np
import ml_dtypes

import concourse.bass as bass
import concourse.bacc as bacc
import concourse.mybir as mybir
import concourse.tile as tile
from concourse.tile_rust import add_dep_helper

BF16 = mybir.dt.bfloat16
F32 = mybir.dt.float32
AF = mybir.ActivationFunctionType
OP = mybir.AluOpType
bf16 = ml_dtypes.bfloat16

B, C, H, W = 4, 256, 64, 64
N = H * W            # 4096 pixels
QK = 64
IS = N // 2          # 2048 queries per core
NJT = N // 128       # 32 key tiles
NIB = IS // 512      # 4 i-blocks
NCH = 4              # den allreduce chunks
JCH = NJT // NCH     # 16 key tiles per chunk
N_CORES = 8
REPLICA_GROUPS = [[0, 1], [2, 3], [4, 5], [6, 7]]


def build_program(n_cores: int = N_CORES, enable_asserts: bool = False,
                  use_softplus: bool = False) -> bass.Bass:
    nc = bacc.Bacc(
        "TRN2",
        target_bir_lowering=False,
        debug=False,
        enable_asserts=enable_asserts,
        num_devices=n_cores,
    )

    # Per-core inputs (data differs by core; program is identical).
    # xq/xb/xf hold the two 128-row channel halves side by side:
    # [:, kt*COLS : (kt+1)*COLS] is channel rows kt*128..kt*128+127.
    xq_d = nc.dram_tensor("xq", [128, 2 * IS], BF16, kind="ExternalInput").ap()
    xb_d = nc.dram_tensor("xb", [128, 2 * N], BF16, kind="ExternalInput").ap()
    xf_d = nc.dram_tensor("xf", [128, 2 * IS], F32, kind="ExternalInput").ap()
    pe1q_d = nc.dram_tensor("pe1q", [QK, IS], BF16, kind="ExternalInput").ap()
    # Shared weights (same on all cores).
    pe1_d = nc.dram_tensor("pe1", [QK, N], BF16, kind="ExternalInput").ap()
    wqk_d = nc.dram_tensor("wqk", [128, 256], BF16, kind="ExternalInput").ap()
    # wmlp = wvt | w1t | w2t | bvb
    wmlp_d = nc.dram_tensor("wmlp", [128, 1792], BF16, kind="ExternalInput").ap()
    bcols_d = nc.dram_tensor("bcols", [128, 4], F32, kind="ExternalInput").ap()

    y_d = nc.dram_tensor("y", [C, IS], F32, kind="ExternalOutput").ap()

    with tile.TileContext(nc) as tc:
        with (
            tc.tile_pool(name="const", bufs=1) as cpool,
            tc.tile_pool(name="qk", bufs=1) as qkpool,
            tc.tile_pool(name="outsb", bufs=1) as outpool,
            tc.tile_pool(name="den", bufs=1) as denpool,
            tc.tile_pool(name="dram", bufs=1, space="DRAM") as dram,
        ):
            # q-projection inputs first (gate the whole pipeline), then the
            # rest; bulky non-urgent loads go on the gpsimd queue.
            wqk_sb = cpool.tile([128, 256], BF16)
            nc.sync.dma_start(wqk_sb[:], wqk_d[:])

            q_sb = qkpool.tile([QK, IS], BF16)     # q, d on partitions
            k_sb = qkpool.tile([QK, N], BF16)      # k, d on partitions
            vtpool = tc.alloc_tile_pool(name="vt", bufs=1, side="right")
            vt_sb = vtpool.tile([128, NJT * 256], BF16)   # v^T, j on partitions
            den_sb = denpool.tile([128, NJT], F32)
            dsum_sb = denpool.tile([128, NJT], F32)
            rden_sb = denpool.tile([128, NJT], F32)
            out_sb = [outpool.tile([128, IS], BF16, name=f"out_sb{i}") for i in range(2)]

            # ---- Phase 1 + early S: projections interleaved with the first
            # NBOOT S-tiles (computed in [128,1024] PSUM halves so they fit
            # alongside the projection PSUM pools) to start ScalarE's exp
            # stream as early as possible. ----
            NBOOT = 10
            with tc.tile_pool(name="attn", bufs=1) as apool:
                attn_sb = apool.tile([128, NJT * IS], BF16)   # 16 MiB
                with (
                    tc.tile_pool(name="xq", bufs=1) as xqpool,
                    tc.tile_pool(name="xb", bufs=1) as xbpool,
                    tc.tile_pool(name="pe", bufs=1) as pepool,
                    tc.tile_pool(name="psA", bufs=2, space="PSUM") as psA,
                    tc.tile_pool(name="psV", bufs=2, space="PSUM") as psV,
                    tc.tile_pool(name="psS0", bufs=2, space="PSUM") as psS0,
                ):
                    xq_sb = xqpool.tile([128, 2 * IS], BF16)
                    xb_sb = xbpool.tile([128, 2 * N], BF16)
                    pe1q_sb = pepool.tile([QK, IS], BF16)
                    pe1_sb = pepool.tile([QK, N], BF16)
                    for ch in range(2):
                        nc.sync.dma_start(xq_sb[:, bass.ts(ch, IS)],
                                          xq_d[:, bass.ts(ch, IS)])
                    nc.sync.dma_start(pe1q_sb[:], pe1q_d[:])
                    for ch in range(4):
                        nc.sync.dma_start(xb_sb[:, bass.ts(ch, N // 2)],
                                          xb_d[:, bass.ts(ch, N // 2)])
                    for ch in range(2):
                        nc.sync.dma_start(pe1_sb[:, bass.ts(ch, N // 2)],
                                          pe1_d[:, bass.ts(ch, N // 2)])
                    wmlp_sb = cpool.tile([128, 1792], BF16)
                    bcols_sb = cpool.tile([128, 4], F32)
                    nc.gpsimd.dma_start(wmlp_sb[:], wmlp_d[:])
                    nc.gpsimd.dma_start(bcols_sb[:], bcols_d[:])
                    wvt = wmlp_sb[:, 0:512]
                    w1t = wmlp_sb[:, 512:1024]
                    w2t = wmlp_sb[:, 1024:1536]
                    bvb = wmlp_sb[:, 1536:1792]
                    b1c = bcols_sb[:, 0:2]
                    b2c = bcols_sb[:, 2:4]

                    den_h = denpool.tile([128, 2 * NBOOT], F32)

                    def q_proj(ib):
                        sl = bass.ts(ib, 512)
                        ps = psA.tile([QK, 512], F32, name="psa")
                        for kt in range(2):
                            nc.tensor.matmul(ps[:], wqk_sb[:, bass.ts(kt, QK)],
                                             xq_sb[:, ib * 1024 + kt * 512:
                                                   ib * 1024 + (kt + 1) * 512],
                                             start=(kt == 0), stop=(kt == 1))
                        nc.vector.tensor_add(q_sb[:, sl], ps[:], pe1q_sb[:, sl])

                    def k_proj(jb):
                        sl = bass.ts(jb, 512)
                        ps = psA.tile([QK, 512], F32, name="psa")
                        for kt in range(2):
                            nc.tensor.matmul(ps[:], wqk_sb[:, 128 + kt * QK:
                                                           128 + (kt + 1) * QK],
                                             xb_sb[:, kt * N + jb * 512:
                                                   kt * N + (jb + 1) * 512],
                                             start=(kt == 0), stop=(kt == 1))
                        nc.vector.tensor_add(k_sb[:, sl], ps[:], pe1_sb[:, sl])

                    def s_boot(jt):
                        for h2 in range(2):
                            ps0 = psS0.tile([128, 1024], F32, name="pss0")
                            for n2 in range(2):
                                ib = h2 * 2 + n2
                                nc.tensor.matmul(ps0[:, bass.ts(n2, 512)],
                                                 k_sb[:, bass.ts(jt, 128)],
                                                 q_sb[:, bass.ts(ib, 512)],
                                                 start=True, stop=True)
                            nc.scalar.activation(
                                attn_sb[:, jt * IS + h2 * 1024:
                                        jt * IS + (h2 + 1) * 1024],
                                ps0[:], AF.Exp, scale=0.125,
                                accum_out=den_h[:, h2 * NBOOT + jt:
                                                h2 * NBOOT + jt + 1])

                    def v_proj(jt):
                        ps = psV.tile([128, 256], F32, name="psv")
                        for kt in range(2):
                            nc.tensor.matmul(ps[:],
                                             xb_sb[:, kt * N + jt * 128:
                                                   kt * N + (jt + 1) * 128],
                                             wvt[:, bass.ts(kt, 256)],
                                             start=(kt == 0), stop=(kt == 1))
                        nc.vector.tensor_add(vt_sb[:, bass.ts(jt, 256)],
                                             ps[:], bvb[:])

                    for ib in range(NIB):
                        q_proj(ib)
                    k_proj(0)
                    k_proj(1)
                    for jt in range(4):
                        s_boot(jt)
                    k_proj(2)
                    for jt in range(4, 8):
                        s_boot(jt)
                    k_proj(3)
                    for jt in range(8, NBOOT):
                        s_boot(jt)
                    for jb in range(4, N // 512):
                        k_proj(jb)
                    for jt in range(NJT):
                        v_proj(jt)
                    nc.vector.tensor_add(den_sb[:, 0:NBOOT],
                                         den_h[:, 0:NBOOT],
                                         den_h[:, NBOOT:2 * NBOOT])

                # ---- Phase 2: remaining S[j,i] = (k^T q)/8, attnT = exp(S) ----
                with tc.tile_pool(name="psS", bufs=2, space="PSUM") as psS:
                    for jt in range(NBOOT, NJT):
                        ps = psS.tile([128, IS], F32)
                        for ib in range(NIB):
                            nc.tensor.matmul(ps[:, bass.ts(ib, 512)],
                                             k_sb[:, bass.ts(jt, 128)],
                                             q_sb[:, bass.ts(ib, 512)],
                                             start=True, stop=True)
                        nc.scalar.activation(attn_sb[:, bass.ts(jt, IS)], ps[:],
                                             AF.Exp, scale=0.125,
                                             accum_out=den_sb[:, jt:jt + 1])

                # den AllReduce within the core pair (2 chunks, latency hidden),
                # then fold 1/den into v^T
                vtspool = tc.alloc_tile_pool(name="vts", bufs=1)
                vts_sb = vtspool.tile([128, NJT * 256], BF16)  # v^T / den
                for ch in range(NCH):
                    csl = bass.ts(ch, JCH)
                    den_in = dram.tile([128, JCH], F32, name=f"den_in{ch}")
                    den_out = dram.tile([128, JCH], F32, name=f"den_out{ch}")
                    nc.sync.dma_start(den_in[:], den_sb[:, csl])
                    nc.gpsimd.collective_compute(
                        "AllReduce", OP.add,
                        replica_groups=REPLICA_GROUPS,
                        ins=[den_in.opt()], outs=[den_out.opt()],
                    )
                    nc.sync.dma_start(dsum_sb[:, csl], den_out[:])
                    nc.vector.reciprocal(rden_sb[:, csl], dsum_sb[:, csl])
                    for jt in range(ch * JCH, (ch + 1) * JCH):
                        nc.vector.tensor_scalar_mul(vts_sb[:, bass.ts(jt, 256)],
                                                    vt_sb[:, bass.ts(jt, 256)],
                                                    rden_sb[:, jt:jt + 1])
                vtpool.release()

                # ---- Phase 3: out[c,i] = sum_j vts[j,c] * attnT[j,i] ----
                # Two j-half visits so the second den chunk's allreduce hides
                # behind the first half's matmuls.
                # Phase 4's matmuls reuse the 8 out-accumulator PSUM banks
                # (Tile serializes on the read->overwrite dependencies), and
                # visit 2 runs ib-major so each i-block's MLP front (W1 matmul,
                # mish exp/ln on ScalarE) overlaps the remaining attn@v work.
                with (
                    tc.tile_pool(name="xf", bufs=1) as xfpool,
                    tc.tile_pool(name="h", bufs=1) as hpool,
                    tc.tile_pool(name="mtmp", bufs=1) as mpool,
                    tc.tile_pool(name="y", bufs=2) as ypool,
                    tc.tile_pool(name="psO", bufs=1, space="PSUM") as psO,
                ):
                    xf_sb = xfpool.tile([128, 2 * IS], F32)
                    nc.gpsimd.dma_start(xf_sb[:], xf_d[:])
                    h_sb = [hpool.tile([128, IS], BF16, name=f"h_sb{i}")
                            for i in range(2)]
                    pso = {}
                    for mt in range(2):
                        for ib in range(NIB):
                            pso[mt, ib] = psO.tile([128, 512], F32,
                                                   name=f"pso{mt}{ib}")

                    def av_mms(mt, ib, jlo, jhi):
                        for jt in range(jlo, jhi):
                            nc.tensor.matmul(
                                pso[mt, ib][:],
                                vts_sb[:, jt * 256 + mt * 128:
                                       jt * 256 + (mt + 1) * 128],
                                attn_sb[:, jt * IS + ib * 512:
                                        jt * IS + (ib + 1) * 512],
                                start=(jt == 0), stop=(jt == NJT - 1),
                                skip_group_check=True)

                    sp_ts = {}
                    mish_exps, mish_lns = [], []
                    for v in range(NCH - 1):
                        for mt in range(2):
                            for ib in range(NIB):
                                av_mms(mt, ib, v * JCH, (v + 1) * JCH)
                    for ib in range(NIB):
                        for mt in range(2):
                            av_mms(mt, ib, (NCH - 1) * JCH, NJT)
                        for mt in range(2):
                            nc.vector.tensor_copy(
                                out_sb[mt][:, bass.ts(ib, 512)], pso[mt, ib][:])
                        # MLP front for this i-block (reuses the freed banks);
                        # exp lands in h_sb's storage (dead until the final
                        # mish multiply overwrites it)
                        for mt in range(2):
                            ps = pso[mt, ib]
                            for kt in range(2):
                                nc.tensor.matmul(
                                    ps[:],
                                    w1t[:, kt * 256 + mt * 128:
                                        kt * 256 + (mt + 1) * 128],
                                    out_sb[kt][:, bass.ts(ib, 512)],
                                    start=(kt == 0), stop=(kt == 1),
                                    skip_group_check=True)
                            mish_exps.append(nc.scalar.activation(
                                h_sb[mt][:, bass.ts(ib, 512)], ps[:], AF.Exp,
                                bias=b1c[:, mt:mt + 1]))
                    # softplus: sp = ln(1+e), batched so the table set switches
                    # only twice (exp set -> ln set -> tanh/exp set)
                    for ib in range(NIB):
                        for mt in range(2):
                            sp_t = mpool.tile([128, 512], BF16,
                                              name=f"mish_sp{mt}{ib}", bufs=1)
                            ln_i = nc.scalar.activation(
                                sp_t[:], h_sb[mt][:, bass.ts(ib, 512)],
                                AF.Ln, bias=1.0)
                            for e in mish_exps:
                                add_dep_helper(ln_i.ins, e.ins, sync=False,
                                               reason="batch act table sets")
                            mish_lns.append(ln_i)
                            sp_ts[mt, ib] = sp_t

                    # tanh batch (one table switch), h = (h_psum+b1)*tanh(sp)
                    for ib in range(NIB):
                        for mt in range(2):
                            th_t = mpool.tile([128, 512], BF16, name="mish_th",
                                              bufs=2)
                            th_i = nc.scalar.activation(th_t[:],
                                                        sp_ts[mt, ib][:], AF.Tanh)
                            for l in mish_lns:
                                add_dep_helper(th_i.ins, l.ins, sync=False,
                                               reason="batch act table sets")
                            nc.vector.scalar_tensor_tensor(
                                h_sb[mt][:, bass.ts(ib, 512)], pso[mt, ib][:],
                                b1c[:, mt:mt + 1], th_t[:],
                                op0=OP.add, op1=OP.mult)
                    for ib in range(NIB):
                        for mt in range(2):
                            sl = bass.ts(ib, 512)
                            ps = pso[mt, ib]
                            for kt in range(2):
                                nc.tensor.matmul(
                                    ps[:],
                                    w2t[:, kt * 256 + mt * 128:
                                        kt * 256 + (mt + 1) * 128],
                                    h_sb[kt][:, sl],
                                    start=(kt == 0), stop=(kt == 1),
                                    skip_group_check=True)
                            y_sb = ypool.tile([128, 512], F32)
                            nc.vector.scalar_tensor_tensor(
                                y_sb[:], ps[:], b2c[:, mt:mt + 1],
                                xf_sb[:, mt * IS + ib * 512:
                                      mt * IS + (ib + 1) * 512],
                                op0=OP.add, op1=OP.add)
                            nc.sync.dma_start(
                                y_d[mt * 128:(mt + 1) * 128, sl], y_sb[:])
                vtspool.release()
    nc.finalize()
    return nc


def _to_lhsT_sb(w):
    """[256, M] fp32 -> SBUF layout [128, 2*M] bf16: col block kt holds rows
    kt*128..kt*128+127 of w."""
    k, m = w.shape
    assert k == 256
    return np.ascontiguousarray(
        w.reshape(2, 128, m).transpose(1, 0, 2).reshape(128, 2 * m).astype(bf16))


def _bf(a):
    return np.ascontiguousarray(np.asarray(a, dtype=np.float32).astype(bf16))


def _halves(a):
    """[256, X] -> [128, 2*X] with the two 128-row halves side by side."""
    return np.ascontiguousarray(np.concatenate([a[:128], a[128:]], axis=1))


def make_in_maps(x, WQ, bQ, WK, bK, WV, bV, PE, W1, b1, W2, b2, n_cores=N_CORES):
    x = np.asarray(x, dtype=np.float32)
    xf3 = np.ascontiguousarray(x.reshape(B, C, N))
    pef = np.asarray(PE, dtype=np.float32).reshape(QK, N)
    pe1 = _bf(pef + np.asarray(bK, np.float32)[:, None])
    pe1q_full = _bf(pef + np.asarray(bQ, np.float32)[:, None])

    wq = _to_lhsT_sb(np.asarray(WQ, np.float32).T)   # [128, 128]
    wk = _to_lhsT_sb(np.asarray(WK, np.float32).T)
    wmlp = np.concatenate([
        _to_lhsT_sb(np.asarray(WV, np.float32).T),
        _to_lhsT_sb(np.asarray(W1, np.float32).T),
        _to_lhsT_sb(np.asarray(W2, np.float32).T),
        np.broadcast_to(_bf(np.asarray(bV)[None, :]), (128, 256)),
    ], axis=1)
    bcols = np.concatenate([
        np.asarray(b1, np.float32).reshape(2, 128).T,
        np.asarray(b2, np.float32).reshape(2, 128).T,
    ], axis=1)

    shared = {
        "pe1": pe1,
        "wqk": np.ascontiguousarray(np.concatenate([wq, wk], axis=1)),
        "wmlp": np.ascontiguousarray(wmlp),
        "bcols": np.ascontiguousarray(bcols),
    }
    in_maps = []
    for core in range(n_cores):
        s, h = core // 2, core % 2
        isl = slice(h * IS, (h + 1) * IS)
        xb = _bf(xf3[s])
        m = dict(shared)
        m["xb"] = _halves(xb)
        # xq is ib-interleaved: [:, ib*1024+kt*512 : ...] = channel-half kt,
        # query block ib — so the first DMA chunk covers ib 0-1 completely.
        xqs = xb[:, isl]
        m["xq"] = np.ascontiguousarray(np.concatenate(
            [np.concatenate([xqs[:128, ib * 512:(ib + 1) * 512],
                             xqs[128:, ib * 512:(ib + 1) * 512]], axis=1)
             for ib in range(NIB)], axis=1))
        m["xf"] = _halves(xf3[s][:, isl])
        m["pe1q"] = np.ascontiguousarray(pe1q_full[:, isl])
        in_maps.append(m)
    return in_maps


def assemble_output(results, n_cores=N_CORES):
    y = np.empty((B, C, N), dtype=np.float32)
    for s in range(B):
        y[s][:, :IS] = results[2 * s]["y"]
        y[s][:, IS:] = results[2 * s + 1]["y"]
    return y.reshape(B, C, H, W)


_PROG = None


def kernel(**inputs) -> np.ndarray:
    global _PROG
    from concourse.bass_utils import run_bass_kernel_spmd
    if _PROG is None:
        _PROG = build_program(N_CORES)
    in_maps = make_in_maps(**inputs)
    res = run_bass_kernel_spmd(_PROG, in_maps, core_ids=list(range(N_CORES)))
    return assemble_output(res.results)
